# revision 3
# baseline (speedup 1.0000x reference)
"""AttentionSequencePoolingLayer (DIN attention) on 8 trn2 NeuronCores.

Data-parallel over batch: B=2048 -> 256 per core.
Math per (b,t):  att = concat([q,k,q-k,q*k]) @ W1 + b1
  Using row-blocks W1 = [W1a;W1b;W1c;W1d]:
    att = q@(W1a+W1c) + k@(W1b-W1c) + (q*k)@W1d
        = k @ (Bw + diag(q_b) C) + (q_b@A + b1)      (folded per-batch weight)
  h1 = sigmoid(att); h2 = sigmoid(h1@W2+b2); s = h2@W3+b3 (masked); out = s^T K.

Host precomputes (numpy): folded per-batch W_b [64,80], U_b = q_b@A+b1 [80],
keys in feature-major layout for the MLP rhs, and mask-folded keys in
token-major layout for the pooling matmul.  Device does all matmuls/sigmoids.
"""
import numpy as np

import concourse.bacc as bacc
import concourse.bass as bass
import concourse.mybir as mybir
import concourse.tile as tile
from concourse.bass_utils import run_bass_kernel_spmd

B, T, E = 2048, 200, 64
H1, H2 = 80, 40
NCORES = 8
BL = B // NCORES          # 256 batches per core
BT = 32                   # batch tile
NT = BL // BT             # 8 batch tiles
TH = T // 2               # 100-token halves for pooling partitions

_cache = {}

# opt-in profiling knobs (test.py sets these; harness leaves defaults)
TRACE = False
TRACE_KW = {}
LAST_RESULT = None


def _build(b3f: float):
    nc = bacc.Bacc(None, target_bir_lowering=False)
    f32 = mybir.dt.float32

    keysT_d = nc.dram_tensor("keysT", [E, BL, T], f32, kind="ExternalInput")
    knatm_d = nc.dram_tensor("knatm", [TH, BL, 2, E], f32, kind="ExternalInput")
    wfold_d = nc.dram_tensor("wfold", [E, BL, H1], f32, kind="ExternalInput")
    ut_d = nc.dram_tensor("ut", [H1, BL], f32, kind="ExternalInput")
    w2_d = nc.dram_tensor("w2", [H1, H2], f32, kind="ExternalInput")
    w3_d = nc.dram_tensor("w3", [H2, 1], f32, kind="ExternalInput")
    b2_d = nc.dram_tensor("b2c", [H2, 1], f32, kind="ExternalInput")
    out_d = nc.dram_tensor("out", [BL * E], f32, kind="ExternalOutput")

    with tile.TileContext(nc) as tc:
        with (
            tc.tile_pool(name="big", bufs=2) as big,
            tc.tile_pool(name="const", bufs=1) as const,
            tc.tile_pool(name="work", bufs=3) as work,
            tc.tile_pool(name="p1", bufs=2, space=bass.MemorySpace.PSUM) as p1p,
            tc.tile_pool(name="p2", bufs=2, space=bass.MemorySpace.PSUM) as p2p,
            tc.tile_pool(name="pS", bufs=2, space=bass.MemorySpace.PSUM) as pSp,
            tc.tile_pool(name="po", bufs=2, space=bass.MemorySpace.PSUM) as pop,
        ):
            ut_s = const.tile([H1, BL], f32)
            w2_s = const.tile([H1, H2], f32)
            w3_s = const.tile([H2, 1], f32)
            b2_s = const.tile([H2, 1], f32)
            nc.sync.dma_start(ut_s[:], ut_d[:])
            nc.sync.dma_start(w2_s[:], w2_d[:])
            nc.sync.dma_start(w3_s[:], w3_d[:])
            nc.sync.dma_start(b2_s[:], b2_d[:])

            SIG = mybir.ActivationFunctionType.Sigmoid

            for bt in range(NT):
                b0 = bt * BT
                kt = big.tile([E, BT, T], f32, tag="kt")
                kn = big.tile([TH, BT, 2, E], f32, tag="kn")
                wt = big.tile([E, BT, H1], f32, tag="wt")
                nc.sync.dma_start(kt[:], keysT_d[:, b0 : b0 + BT, :])
                nc.sync.dma_start(kn[:], knatm_d[:, b0 : b0 + BT, :, :])
                nc.sync.dma_start(wt[:], wfold_d[:, b0 : b0 + BT, :])

                Sps = pSp.tile([TH, 2, BT], f32, tag="Sps")
                for i in range(BT):
                    g = b0 + i
                    h1psum = p1p.tile([H1, T], f32, tag="h1ps")
                    nc.tensor.matmul(
                        h1psum[:], wt[:, i, :], kt[:, i, :], start=True, stop=True
                    )
                    h1t = work.tile([H1, T], f32, tag="h1t")
                    nc.scalar.activation(
                        h1t[:], h1psum[:], SIG, bias=ut_s[:, g : g + 1]
                    )
                    h2psum = p2p.tile([H2, T], f32, tag="h2ps")
                    nc.tensor.matmul(
                        h2psum[:], w2_s[:], h1t[:], start=True, stop=True
                    )
                    h2t = work.tile([H2, T], f32, tag="h2t")
                    nc.scalar.activation(
                        h2t[:], h2psum[:], SIG, bias=b2_s[:, 0:1]
                    )
                    for h in range(2):
                        nc.tensor.matmul(
                            Sps[:, h, i : i + 1],
                            h2t[:, h * TH : (h + 1) * TH],
                            w3_s[:],
                            start=True,
                            stop=True,
                        )

                # scores + b3 (mask already folded into kn on host)
                STm = work.tile([TH, 2, BT], f32, tag="STm")
                nc.scalar.add(STm[:], Sps[:], b3f)

                for i in range(BT):
                    if i % 8 == 0:
                        pout = pop.tile([1, 8 * E], f32, tag="pout")
                    for h in range(2):
                        nc.tensor.matmul(
                            pout[0:1, (i % 8) * E : (i % 8 + 1) * E],
                            STm[:, h, i : i + 1],
                            kn[:, i, h, :],
                            start=(h == 0),
                            stop=(h == 1),
                        )
                    if i % 8 == 7:
                        orow = work.tile([1, 8 * E], f32, tag="orow")
                        nc.scalar.copy(orow[:], pout[:])
                        g0 = b0 + i - 7
                        nc.sync.dma_start(
                            out_d[g0 * E : (g0 + 8) * E], orow[:]
                        )

    nc.compile()
    return nc


def kernel(query, keys, keys_length, W1, b1, W2, b2, W3, b3):
    query = np.asarray(query, np.float32)
    keys = np.asarray(keys, np.float32)
    keys_length = np.asarray(keys_length, np.int32)
    W1 = np.asarray(W1, np.float32); b1 = np.asarray(b1, np.float32)
    W2 = np.asarray(W2, np.float32); b2 = np.asarray(b2, np.float32)
    W3 = np.asarray(W3, np.float32); b3 = np.asarray(b3, np.float32)

    A = W1[0:E] + W1[2 * E : 3 * E]          # q coeff
    Bw = W1[E : 2 * E] - W1[2 * E : 3 * E]   # k coeff
    C = W1[3 * E : 4 * E]                    # q*k coeff

    q2 = query[:, 0, :]                      # [B, E]
    U = q2 @ A + b1                          # [B, H1]
    # folded per-batch first-layer weight: [B, E, H1]
    Wf = Bw[None, :, :] + q2[:, :, None] * C[None, :, :]
    mask = (np.arange(T)[None, :] < keys_length).astype(np.float32)  # [B, T]
    kmask = keys * mask[:, :, None]          # [B, T, E]

    b3f = float(b3.reshape(-1)[0])
    if "nc" not in _cache:
        _cache["nc"] = _build(b3f)
    nc = _cache["nc"]

    in_maps = []
    for c in range(NCORES):
        s = slice(c * BL, (c + 1) * BL)
        kc = keys[s]                                          # [BL, T, E]
        in_maps.append({
            "keysT": np.ascontiguousarray(kc.transpose(2, 0, 1)),
            "knatm": np.ascontiguousarray(
                kmask[s].reshape(BL, 2, TH, E).transpose(2, 0, 1, 3)
            ),
            "wfold": np.ascontiguousarray(Wf[s].transpose(1, 0, 2)),
            "ut": np.ascontiguousarray(U[s].T),
            "w2": W2,
            "w3": W3,
            "b2c": b2.reshape(H2, 1),
        })

    res = run_bass_kernel_spmd(
        nc, in_maps, list(range(NCORES)), trace=TRACE, **TRACE_KW
    )
    global LAST_RESULT
    LAST_RESULT = res
    outs = [np.asarray(r["out"]).reshape(BL, E) for r in res.results]
    return np.concatenate(outs, 0).reshape(B, 1, E).astype(np.float32)



# revision 21
# speedup vs baseline: 3.8678x; 3.8678x over previous
"""AttentionSequencePoolingLayer (DIN attention) on 8 trn2 NeuronCores.

Data-parallel over batch: B=2048 -> 256 per core, processed as 64
"supergroups" of 4 batches (2 groups of 2).

Math per (b,t):  att = concat([q,k,q-k,q*k]) @ W1 + b1
  With W1 = [W1a;W1b;W1c;W1d] row blocks:
    att = k@(W1b-W1c) + (q*k)@W1d + (q@(W1a+W1c) + b1)
        = k@Wf_b + U_b          with Wf_b = (W1b-W1c) + diag(q_b)@W1d
  Device: rhs = [k; 1] (65 rows, ones row baked into the HBM keys layout),
  lhsT = [Wf_b; U_b] (65x80 per batch) -> the U bias rides the matmul, so
  sigmoid needs no per-batch bias and activations merge into big instrs.
  h1 = sigmoid(.); h2 = sigmoid(W2^T h1 + b2); scores via a block W3 that
  broadcasts s_A to partitions 0..63 and s_B to 64..127; pooling is one
  fused DVE tensor_tensor_reduce per group against even/odd-stacked masked
  keys, with b3*colsum(mK) as the reduction init (mask pre-folded into keys
  host-side; padded positions contribute 0 regardless of their score).
"""
import numpy as np
import ml_dtypes

import concourse.bacc as bacc
import concourse.bass as bass
import concourse.mybir as mybir
import concourse.tile as tile
from concourse.bass_utils import run_bass_kernel_spmd

B, T, E = 2048, 200, 64
H1, H2 = 80, 40
NCORES = 8
BL = B // NCORES          # 256 batches per core
NG = BL // 2              # 128 groups of 2 batches
NSG = BL // 4             # 64 supergroups of 4 batches

BF16 = ml_dtypes.bfloat16

_cache = {}

# opt-in profiling knobs (test.py sets these; harness leaves defaults)
TRACE = False
TRACE_KW = {}
LAST_RESULT = None


def _build(b3f: float):
    nc = bacc.Bacc(None, target_bir_lowering=False)
    f32 = mybir.dt.float32
    f32r = mybir.dt.float32r
    bf16 = mybir.dt.bfloat16
    SIG = mybir.ActivationFunctionType.Sigmoid
    MULT = mybir.AluOpType.mult
    ADD = mybir.AluOpType.add

    kaug_d = nc.dram_tensor("kaug", [E + 1, BL * T], bf16, kind="ExternalInput")
    kst_d = nc.dram_tensor("kst", [128, (BL // 2) * T], bf16, kind="ExternalInput")
    wfu_d = nc.dram_tensor("wfu", [E + 1, BL * H1], bf16, kind="ExternalInput")
    w2e_d = nc.dram_tensor("w2e", [H1, 64], bf16, kind="ExternalInput")
    w2b_d = nc.dram_tensor("w2b", [H1, 64], bf16, kind="ExternalInput")
    w3blk_d = nc.dram_tensor("w3blk", [128, 128], f32r, kind="ExternalInput")
    b2c_d = nc.dram_tensor("b2c", [128, 1], f32, kind="ExternalInput")
    acc_d = nc.dram_tensor("acc", [128, NG], f32, kind="ExternalOutput")

    with tile.TileContext(nc) as tc:
        with (
            tc.tile_pool(name="const", bufs=1) as const,
            tc.tile_pool(name="keys", bufs=3) as keys_p,
            tc.tile_pool(name="wf", bufs=3) as wf_p,
            tc.tile_pool(name="act", bufs=3) as act_p,
            tc.tile_pool(name="p1", bufs=2, space=bass.MemorySpace.PSUM) as p1p,
            tc.tile_pool(name="p2", bufs=2, space=bass.MemorySpace.PSUM) as p2p,
            tc.tile_pool(name="p3", bufs=2, space=bass.MemorySpace.PSUM) as p3p,
        ):
            w2e_s = const.tile([H1, 64], bf16)
            w2b_s = const.tile([H1, 64], bf16)
            w3blk_s = const.tile([128, 128], f32r)
            b2c_s = const.tile([128, 1], f32)
            acc_s = const.tile([128, NG], f32)
            nc.sync.dma_start(w2e_s[:], w2e_d[:])
            nc.sync.dma_start(w2b_s[:], w2b_d[:])
            nc.sync.dma_start(w3blk_s[:], w3blk_d[:])
            nc.sync.dma_start(b2c_s[:], b2c_d[:])

            for j in range(NSG):
                kt = keys_p.tile([E + 1, 4 * T], bf16, tag="kt")
                k2 = keys_p.tile([128, 2 * T], bf16, tag="k2")
                wf = wf_p.tile([E + 1, 4 * H1], bf16, tag="wf")
                nc.sync.dma_start(kt[:], kaug_d[:, j * 4 * T : (j + 1) * 4 * T])
                nc.sync.dma_start(k2[:], kst_d[:, j * 2 * T : (j + 1) * 2 * T])
                nc.sync.dma_start(wf[:], wfu_d[:, j * 4 * H1 : (j + 1) * 4 * H1])

                # L1: att = [Wf_b; U_b]^T @ [k; 1] per batch.  PSUM tile is
                # 2 full banks; batches at cols 0,200,512,712 so each matmul
                # stays inside one bank.
                # batch order [0,2,1,3]: evens land in h1 cols 0..399, odds
                # in 400..799, so both L2 matmuls get contiguous rhs slices
                p1 = p1p.tile([H1, 1024], f32, tag="p1")
                for b, bb in enumerate((0, 2, 1, 3)):
                    c0 = b * T + (b // 2) * 112
                    nc.tensor.matmul(
                        p1[:, c0 : c0 + T],
                        wf[:, bb * H1 : (bb + 1) * H1],
                        kt[:, bb * T : (bb + 1) * T],
                        start=True,
                        stop=True,
                    )
                h1 = act_p.tile([H1, 4 * T], bf16, tag="h1")
                p1v = p1[:, :].rearrange("p (a b) -> p a b", a=2)[:, :, 0:400]
                h1v = h1[:, :].rearrange("p (a b) -> p a b", a=2)
                nc.scalar.activation(h1v, p1v, SIG)

                # L2: h2 stacked per group: batch even -> rows 0..63 (w2e has
                # zero cols 40..63), batch odd -> rows 64..103.  fp32r with
                # 400-col free dim runs at full PE rate; rhs gathers the two
                # even (resp. odd) batches via a strided view of h1.
                p2 = p2p.tile([128, 2 * T], f32, tag="p2", padded_shape=[128, 512])
                nc.tensor.matmul(
                    p2[0:64, :],
                    w2e_s[:],
                    h1[:, 0 : 2 * T],
                    start=True,
                    stop=True,
                )
                nc.tensor.matmul(
                    p2[64:128, :],
                    w2b_s[:],
                    h1[:, 2 * T : 4 * T],
                    start=True,
                    stop=True,
                    tile_position=(0, 64),
                )
                h2 = act_p.tile([128, 2 * T], f32r, tag="h2")
                nc.scalar.activation(h2[:], p2[:], SIG, bias=b2c_s[:, 0:1])

                # L3: scores broadcast: rows 0..63 = s_even, 64..127 = s_odd
                p3 = p3p.tile([128, 2 * T], f32, tag="p3", padded_shape=[128, 512])
                nc.tensor.matmul(
                    p3[:, :],
                    w3blk_s[:],
                    h2[:, :],
                    start=True,
                    stop=True,
                )

                # pooling: acc[:, g] = sum_t (s + b3) * mK  (fused DVE,
                # custom ant-dve op; padded positions have mK == 0)
                sc = act_p.tile([128, 2 * T], bf16, tag="sc")
                for gp in range(2):
                    g = 2 * j + gp
                    nc.vector.affine_mul_reduce(
                        out=sc[:, gp * T : (gp + 1) * T],
                        accum_out=acc_s[:, g : g + 1],
                        in0=p3[:, gp * T : (gp + 1) * T],
                        in1=k2[:, gp * T : (gp + 1) * T],
                        scale=1.0,
                        bias=b3f,
                    )

            nc.sync.dma_start(acc_d[:], acc_s[:])

    nc.compile()
    return nc


def kernel(query, keys, keys_length, W1, b1, W2, b2, W3, b3):
    query = np.asarray(query, np.float32)
    keys = np.asarray(keys, np.float32)
    keys_length = np.asarray(keys_length, np.int32)
    W1 = np.asarray(W1, np.float32); b1 = np.asarray(b1, np.float32)
    W2 = np.asarray(W2, np.float32); b2 = np.asarray(b2, np.float32)
    W3 = np.asarray(W3, np.float32); b3 = np.asarray(b3, np.float32)

    A = W1[0:E] + W1[2 * E : 3 * E]          # q coeff
    Bw = W1[E : 2 * E] - W1[2 * E : 3 * E]   # k coeff
    C = W1[3 * E : 4 * E]                    # q*k coeff

    q2 = query[:, 0, :]                      # [B, E]
    U = q2 @ A + b1                          # [B, H1]
    # folded per-batch first-layer weight + U row: [B, 65, H1]
    Wf = Bw[None, :, :] + q2[:, :, None] * C[None, :, :]
    wfu_all = np.concatenate([Wf, U[:, None, :]], axis=1).astype(BF16)
    mask = (np.arange(T)[None, :] < keys_length).astype(np.float32)  # [B, T]
    mk_all = (keys * mask[:, :, None]).astype(BF16)                  # [B, T, E]
    b3f = float(b3.reshape(-1)[0])
    if _cache.get("b3f") != b3f:
        _cache["nc"] = _build(b3f)
        _cache["b3f"] = b3f
    nc = _cache["nc"]

    w2e = np.zeros((H1, 64), np.float32); w2e[:, 0:H2] = W2
    w3blk = np.zeros((128, 128), np.float32)
    w3blk[0:H2, 0:64] = np.broadcast_to(W3, (H2, 64))
    w3blk[64 : 64 + H2, 64:128] = np.broadcast_to(W3, (H2, 64))
    b2c = np.zeros((128, 1), np.float32)
    b2c[0:H2, 0] = b2; b2c[64 : 64 + H2, 0] = b2

    in_maps = []
    for c in range(NCORES):
        s = slice(c * BL, (c + 1) * BL)
        mk = mk_all[s]                                   # [BL, T, E] bf16
        kfm = np.ascontiguousarray(
            mk.transpose(2, 0, 1).reshape(E, BL * T))
        kaug = np.concatenate(
            [kfm, np.ones((1, BL * T), BF16)], axis=0)   # [65, BL*T]
        kst = np.concatenate(
            [mk[0::2].transpose(2, 0, 1).reshape(E, (BL // 2) * T),
             mk[1::2].transpose(2, 0, 1).reshape(E, (BL // 2) * T)],
            axis=0)                                      # [128, BL/2*T]
        wfu = np.ascontiguousarray(
            wfu_all[s].transpose(1, 0, 2).reshape(E + 1, BL * H1))
        in_maps.append({
            "kaug": np.ascontiguousarray(kaug),
            "kst": np.ascontiguousarray(kst),
            "wfu": wfu,
            "w2e": w2e.astype(BF16),
            "w2b": w2e.astype(BF16),
            "w3blk": w3blk,
            "b2c": b2c,
        })

    res = run_bass_kernel_spmd(
        nc, in_maps, list(range(NCORES)), trace=TRACE, **TRACE_KW
    )
    global LAST_RESULT
    LAST_RESULT = res
    outs = []
    for r in res.results:
        acc = np.asarray(r["acc"], np.float32)           # [128, NG]
        outs.append(acc.reshape(2, E, NG).transpose(2, 0, 1).reshape(BL, E))
    return np.concatenate(outs, 0).reshape(B, 1, E).astype(np.float32)
